# revision 25
# baseline (speedup 1.0000x reference)
"""Causal self-attention with relative position bias, 8-way batch-parallel
across NeuronCores. Self-contained: takes full inputs, returns full output.

Per-core layout strategy:
  - host feeds transposed weights (c-major) so every matmul operand already
    has the contraction dim in partitions; no transpose loads on device
  - rel bias: S2 = q @ R_rev^T per head, then a sheared (diagonal) SBUF->SBUF
    DMA materializes rel[i, j] = S2[i, i-j]; pad columns preloaded with -1e30
    make the causal mask fall out of exp() for free
  - softmax without max-subtraction (scores bounded well inside fp32 exp range)
  - att (bf16) transposed per 128x128 block on the PE, batched 4-wide through
    PSUM so evictions are few and large
  - fp32r (full-rate fp32) for the qkv projection; bf16 for the score/AV path
"""

import numpy as np

B, T, C, NH = 8, 512, 512, 8
HD = C // NH  # 64
N_CORES = 8
EPS = 1e-5
NEG = -1.0e30

_CACHE = {}


def _build_nc(reps: int = 1):
    from contextlib import ExitStack

    import concourse.bass as bass
    import concourse.mybir as mybir
    import concourse.tile as tile
    from concourse import bacc
    from concourse.masks import make_identity

    f32 = mybir.dt.float32
    f32r = mybir.dt.float32r
    bf16 = mybir.dt.bfloat16
    Alu = mybir.AluOpType
    Act = mybir.ActivationFunctionType

    nc = bacc.Bacc("TRN2")

    xT = nc.declare_dram_parameter("xT", [C, T], f32r, isOutput=False)
    w_attnT = nc.declare_dram_parameter("w_attnT", [C, 3 * C], f32r, isOutput=False)
    b_attn = nc.declare_dram_parameter("b_attn", [1, 3 * C], f32, isOutput=False)
    w_projT = nc.declare_dram_parameter("w_projT", [C, C], bf16, isOutput=False)
    b_proj = nc.declare_dram_parameter("b_proj", [1, C], f32, isOutput=False)
    gbT = nc.declare_dram_parameter("gbT", [C, 4], f32, isOutput=False)
    rT_rev = nc.declare_dram_parameter("rT_rev", [C, T], bf16, isOutput=False)
    y = nc.declare_dram_parameter("y", [T, C], f32, isOutput=True)

    with tile.TileContext(nc) as tc, ExitStack() as ctx:
        const = ctx.enter_context(tc.tile_pool(name="const", bufs=1))
        qk_nat = ctx.enter_context(tc.tile_pool(name="qk_nat", bufs=4))
        qkT = ctx.enter_context(tc.tile_pool(name="qkT", bufs=1))
        stats = ctx.enter_context(tc.tile_pool(name="stats", bufs=2))
        e_pool = ctx.enter_context(tc.tile_pool(name="e_pool", bufs=2))
        rel_pool = ctx.enter_context(tc.tile_pool(name="rel_pool", bufs=2))
        att_pool = ctx.enter_context(tc.tile_pool(name="att_pool", bufs=2))
        attT_pool = ctx.enter_context(tc.tile_pool(name="attT_pool", bufs=3))
        y_pool = ctx.enter_context(tc.tile_pool(name="y_pool", bufs=1))
        out_pool = ctx.enter_context(tc.tile_pool(name="out_pool", bufs=2))
        s2_ps = ctx.enter_context(tc.tile_pool(name="s2_ps", bufs=2, space="PSUM"))
        sc_ps = ctx.enter_context(tc.tile_pool(name="sc_ps", bufs=3, space="PSUM"))
        tp_ps = ctx.enter_context(tc.tile_pool(name="tp_ps", bufs=2, space="PSUM"))
        av_ps = ctx.enter_context(tc.tile_pool(name="av_ps", bufs=1, space="PSUM"))

        # ---- constant loads ------------------------------------------------
        # xT/waT chunked per c-tile on both HWDGE queues so qkv can start
        # before the tail chunks land; everything not needed early deferred
        xT_d = xT.rearrange("(ct p) t -> p ct t", p=128)
        waT_d = w_attnT.rearrange("(ct p) t -> p ct t", p=128)
        xT_sb = const.tile([128, 4, T], f32r)
        waT_sb = const.tile([128, 4, 3 * C], f32r)
        for ct in range(4):
            eng = nc.sync if ct % 2 == 0 else nc.scalar
            eng.dma_start(xT_sb[:, ct, :], xT_d[:, ct, :])
            eng.dma_start(waT_sb[:, ct, :], waT_d[:, ct, :])
        ba_sb = const.tile([128, 3 * C], f32)
        nc.sync.dma_start(ba_sb[:], b_attn[:, :].to_broadcast((128, 3 * C)))
        gb_sb = const.tile([128, 4, 4], f32)
        nc.sync.dma_start(gb_sb[:], gbT.rearrange("(ct p) g -> p ct g", p=128))
        rT_sb = const.tile([128, 4, T], bf16)
        nc.scalar.dma_start(rT_sb[:], rT_rev.rearrange("(ct p) t -> p ct t", p=128))
        wpT_sb = const.tile([128, 4, C], bf16)
        nc.sync.dma_start(wpT_sb[:], w_projT.rearrange("(ct p) t -> p ct t", p=128))
        bp_sb = const.tile([128, C], f32)
        nc.sync.dma_start(bp_sb[:], b_proj[:, :].to_broadcast((128, C)))

        ident_b = const.tile([128, 128], bf16)
        make_identity(nc, ident_b[:, :])

        for _rep in range(reps):
            # ---- qkv projection -------------------------------------------
            # q, k natural [t, c] f32; v natural [t, (h d)] bf16
            q_nat = [None] * 4
            k_nat = [None] * 4
            v_sb = [None] * 4
            for tt in range(4):
                for s in range(3):
                    ps = sc_ps.tile([128, 512], f32, name="qkv_ps", tag="sc")
                    for ct in range(4):
                        nc.tensor.matmul(
                            ps[:, :],
                            xT_sb[:, ct, tt * 128 : (tt + 1) * 128],
                            waT_sb[:, ct, s * 512 : (s + 1) * 512],
                            start=(ct == 0),
                            stop=(ct == 3),
                        )
                    bias_b = ba_sb[:, s * 512 : (s + 1) * 512]
                    if s < 2:
                        dst = qk_nat.tile(
                            [128, 512], f32, name="qn" if s == 0 else "kn",
                            tag="qn" if s == 0 else "kn",
                        )
                        nc.vector.scalar_tensor_tensor(
                            dst[:, :], ps[:, :], 1.0, bias_b, Alu.mult, Alu.add
                        )
                        if s == 0:
                            q_nat[tt] = dst
                        else:
                            k_nat[tt] = dst
                    else:
                        vt = const.tile([128, 8, HD], bf16, name="v_sb", tag=f"v{tt}")
                        bias_b3 = bias_b.rearrange("p (h d) -> p h d", d=HD)
                        nc.vector.scalar_tensor_tensor(
                            vt[:, :, :],
                            ps[:, :].rearrange("p (h d) -> p h d", d=HD),
                            1.0,
                            bias_b3,
                            Alu.mult,
                            Alu.add,
                        )
                        v_sb[tt] = vt

            # ---- layernorm + transpose of q, k ----------------------------
            # qT/kT: [c, t] bf16; gamma/beta applied on the transposed side.
            # rstd via DVE-only Newton rsqrt (bit-trick seed) batched over all
            # 8 row-tiles, so ACT only ever runs Exp/Identity/Copy (one LUT
            # table load for the whole kernel).
            i32 = mybir.dt.int32
            qT_sb = qkT.tile([128, 4, T], bf16)
            kT_sb = qkT.tile([128, 4, T], bf16)
            mv_all = stats.tile([128, 8, 2], f32, name="mv_all", tag="mv_all")
            for s in range(2):
                nat = q_nat if s == 0 else k_nat
                for tt in range(4):
                    st6 = stats.tile([128, 6], f32, name="st6", tag="st6")
                    nc.vector.bn_stats(st6[:, :], nat[tt][:, :])
                    nc.vector.bn_aggr(mv_all[:, s * 4 + tt, :], st6[:, :])
            vpe = stats.tile([128, 8], f32, name="vpe", tag="vpe")
            nc.vector.tensor_scalar(vpe[:, :], mv_all[:, :, 1], EPS, None, Alu.add)
            xa = stats.tile([128, 8], f32, name="xa", tag="xa")
            xb = stats.tile([128, 8], f32, name="xb", tag="xb")
            nc.vector.tensor_scalar(
                xb[:, :].bitcast(i32), vpe[:, :].bitcast(i32),
                1, None, Alu.logical_shift_right,
            )
            nc.vector.tensor_scalar(
                xa[:, :].bitcast(i32), xb[:, :].bitcast(i32),
                -1, 0x5F3759DF, Alu.mult, Alu.add,
            )
            cur, nxt = xa, xb
            for _it in range(3):
                n1 = stats.tile([128, 8], f32, name="n1", tag=f"n1_{_it}")
                nc.vector.tensor_tensor(n1[:, :], cur[:, :], cur[:, :], Alu.mult)
                n2 = stats.tile([128, 8], f32, name="n2", tag=f"n2_{_it}")
                nc.vector.scalar_tensor_tensor(
                    n2[:, :], n1[:, :], 1.0, vpe[:, :], Alu.mult, Alu.mult
                )
                n3 = stats.tile([128, 8], f32, name="n3", tag=f"n3_{_it}")
                nc.vector.tensor_scalar(n3[:, :], n2[:, :], -0.5, 1.5, Alu.mult, Alu.add)
                nc.vector.tensor_tensor(nxt[:, :], cur[:, :], n3[:, :], Alu.mult)
                cur, nxt = nxt, cur
            rstd_all = cur
            nmr_all = stats.tile([128, 8], f32, name="nmr_all", tag="nmr_all")
            nc.vector.scalar_tensor_tensor(
                nmr_all[:, :], mv_all[:, :, 0], -1.0, rstd_all[:, :],
                Alu.mult, Alu.mult,
            )
            for s in range(2):
                nat = q_nat if s == 0 else k_nat
                dstT = qT_sb if s == 0 else kT_sb
                gsl = gb_sb[:, :, 2 * s : 2 * s + 1]  # gamma [128, 4, 1]
                bsl = gb_sb[:, :, 2 * s + 1 : 2 * s + 2]  # beta
                for tt in range(4):
                    idx = s * 4 + tt
                    ln = stats.tile([128, 512], bf16, name="ln", tag="ln")
                    nc.scalar.activation(
                        ln[:, :], nat[tt][:, :], Act.Identity,
                        bias=nmr_all[:, idx : idx + 1],
                        scale=rstd_all[:, idx : idx + 1],
                    )
                    tp4 = tp_ps.tile([128, 512], bf16, name="tp4_qk", tag="tp")
                    for ct in range(4):
                        nc.tensor.transpose(
                            tp4[:, ct * 128 : (ct + 1) * 128],
                            ln[:, ct * 128 : (ct + 1) * 128],
                            ident_b[:, :],
                        )
                    # dst[:, ct, tt-block] = tp4[:, ct-block] * gamma + beta
                    for ct in range(4):
                        nc.scalar.activation(
                            dstT[:, ct, tt * 128 : (tt + 1) * 128],
                            tp4[:, ct * 128 : (ct + 1) * 128],
                            Act.Identity,
                            bias=bsl[:, ct, :], scale=gsl[:, ct, :],
                        )

            # ---- attention per head ---------------------------------------
            y_nat = y_pool.tile([128, 4, C], bf16)  # [t, c], per (head, qb)
            for h in range(NH):
                ct_h = h // 2
                p0 = (h % 2) * 64
                q_h = qT_sb[:, ct_h, :][p0 : p0 + 64, :]
                k_h = kT_sb[:, ct_h, :][p0 : p0 + 64, :]
                r_h = rT_sb[:, ct_h, :][p0 : p0 + 64, :]

                attT_all = attT_pool.tile([128, 4, T], bf16, name="attT", tag="attT")
                sums = stats.tile([128, 4], f32, name="sums", tag="sums")

                rels = [None] * 4
                for qb in range(4):
                    W = 128 * (qb + 1)
                    # S2[i, u'] = q_i . R_{W-1-u'} (u reversed via host table)
                    s2 = s2_ps.tile([128, 512], f32, name="s2t", tag="s2")
                    nc.tensor.matmul(
                        s2[:, :W],
                        q_h[:, qb * 128 : (qb + 1) * 128],
                        r_h[:, T - W : T],
                        start=True,
                        stop=True,
                    )
                    ev = e_pool.tile([128, W + 128], bf16, name="ev", tag=f"e{qb}")
                    nc.gpsimd.memset(ev[:, W : W + 128], NEG)
                    if qb in (0, 3):
                        nc.scalar.copy(ev[:, 0:W], s2[:, :W])
                    else:
                        nc.vector.tensor_copy(ev[:, 0:W], s2[:, :W])

                    # sheared read: rel[p, j] = ev[p, 127 - p + j]
                    rel = rel_pool.tile([128, W], bf16, name="rel", tag=f"rel{qb}")
                    L = ev.tensor.shape[-1]
                    src = bass.AP(ev.tensor, ev.offset + 127, [[L - 1, 128], [1, W]])
                    nc.scalar.dma_start(rel[:, :], src)
                    rels[qb] = rel

                for qb in range(4):
                    W = 128 * (qb + 1)
                    sc = sc_ps.tile([128, 512], f32, name="sct", tag="sc")
                    nc.tensor.matmul(
                        sc[:, :W],
                        q_h[:, qb * 128 : (qb + 1) * 128],
                        k_h[:, 0:W],
                        start=True,
                        stop=False,
                    )
                    nc.tensor.matmul(
                        sc[:, :W], ident_b[:, :], rels[qb][:, :],
                        start=False, stop=True,
                    )
                    att = att_pool.tile([128, W], bf16, name="att", tag=f"att{qb}")
                    nc.scalar.activation(
                        att[:, :], sc[:, :W], Act.Exp,
                        scale=0.125, accum_out=sums[:, qb : qb + 1],
                    )
                    tp4 = tp_ps.tile([128, 512], bf16, name="tp4_att", tag="tp")
                    for jb in range(qb + 1):
                        nc.tensor.transpose(
                            tp4[:, jb * 128 : (jb + 1) * 128],
                            att[:, jb * 128 : (jb + 1) * 128],
                            ident_b[:, :],
                        )
                    src3 = tp4[:, 0 : (qb + 1) * 128].rearrange(
                        "p (jb i) -> p jb i", i=128
                    )
                    dst3 = attT_all[:, 0 : qb + 1, qb * 128 : (qb + 1) * 128]
                    nc.vector.tensor_copy(dst3, src3)

                rec = stats.tile([128, 4], f32, name="rec", tag="rec")
                nc.vector.reciprocal(rec[:, :], sums[:, :])

                # av[i, d] accumulated per qb into one PSUM tile, one evict
                av4 = av_ps.tile([128, 4, HD], f32, name="av_ps", tag="av")
                for qb in range(4):
                    for jb in range(qb + 1):
                        nc.tensor.matmul(
                            av4[:, qb, :],
                            attT_all[:, jb, qb * 128 : (qb + 1) * 128],
                            v_sb[jb][:, h, :],
                            start=(jb == 0),
                            stop=(jb == qb),
                        )
                nc.vector.tensor_tensor(
                    y_nat[:, :, h * HD : (h + 1) * HD],
                    av4[:, :, :],
                    rec[:, :].unsqueeze(2).to_broadcast((128, 4, HD)),
                    Alu.mult,
                )

            # ---- transpose y for the projection ---------------------------
            yT_sb = y_pool.tile([128, 4, T], bf16)  # [c, t]
            for tt in range(4):
                tp4 = tp_ps.tile([128, 512], bf16, name="tp4_y", tag="tp")
                for ct in range(4):
                    nc.tensor.transpose(
                        tp4[:, ct * 128 : (ct + 1) * 128],
                        y_nat[:, tt, ct * 128 : (ct + 1) * 128],
                        ident_b[:, :],
                    )
                nc.vector.tensor_copy(
                    yT_sb[:, :, tt * 128 : (tt + 1) * 128],
                    tp4[:, :].rearrange("p (ct t) -> p ct t", t=128),
                )

            # ---- output projection ----------------------------------------
            for tt in range(4):
                ps = sc_ps.tile([128, 512], f32, name="proj_ps", tag="sc")
                for ct in range(4):
                    nc.tensor.matmul(
                        ps[:, :],
                        yT_sb[:, ct, tt * 128 : (tt + 1) * 128],
                        wpT_sb[:, ct, :],
                        start=(ct == 0),
                        stop=(ct == 3),
                    )
                ob = out_pool.tile([128, 512], f32, name="ob", tag="ob")
                nc.vector.scalar_tensor_tensor(
                    ob[:, :], ps[:, :], 1.0, bp_sb[:, :], Alu.mult, Alu.add,
                )
                nc.sync.dma_start(y[tt * 128 : (tt + 1) * 128, :], ob[:, :])

    nc.compile()
    return nc


def _f32r_round(a):
    bits = np.ascontiguousarray(a.astype(np.float32)).view(np.uint32)
    bits = (bits + np.uint32(0x800)) & np.uint32(0xFFFFF000)
    return bits.view(np.float32)


def _prep_maps(inputs):
    import ml_dtypes

    bf = ml_dtypes.bfloat16
    x = np.asarray(inputs["x"], np.float32)
    gbT = np.ascontiguousarray(
        np.stack(
            [
                np.asarray(inputs["q_gamma"], np.float32),
                np.asarray(inputs["q_beta"], np.float32),
                np.asarray(inputs["k_gamma"], np.float32),
                np.asarray(inputs["k_beta"], np.float32),
            ],
            axis=1,
        )
    )
    shared = {
        "w_attnT": _f32r_round(np.asarray(inputs["w_attn"], np.float32).T),
        "b_attn": np.asarray(inputs["b_attn"], np.float32).reshape(1, -1),
        "w_projT": np.ascontiguousarray(
            np.asarray(inputs["w_proj"], np.float32).T
        ).astype(bf),
        "b_proj": np.asarray(inputs["b_proj"], np.float32).reshape(1, -1),
        "gbT": gbT,
        "rT_rev": np.ascontiguousarray(
            np.asarray(inputs["rel_emb"], np.float32)[::-1].T
        ).astype(bf),
    }
    return [dict(shared, xT=_f32r_round(x[b].T)) for b in range(N_CORES)]


def kernel(**inputs):
    from concourse.bass_utils import run_bass_kernel_spmd

    if "nc" not in _CACHE:
        _CACHE["nc"] = _build_nc()
    nc = _CACHE["nc"]
    in_maps = _prep_maps(inputs)
    res = run_bass_kernel_spmd(nc, in_maps, core_ids=list(range(N_CORES)))
    return np.stack([res.results[b]["y"] for b in range(N_CORES)], axis=0)
